# revision 32
# baseline (speedup 1.0000x reference)
"""Trainium2 Bass kernel for nn_CrossAttention (B=4, S=1024, D=512, H=8).

Sharding: 8 cores = batch (4) x head-group (2 groups of 4 heads).
Each core computes a partial [S, E] output over its 256 feature dims;
the host sums the two partials per batch and adds the bias.

v3 design (vs v2 baseline, which was ACT/exp-bound at ~62us):
  - temb removed from the device projection: the host precomputes
    kv_temb = W_slice @ temb (tiny) and ships it inside `tny` twice:
    as a [128, 128]-per-c logits lhs whose only nonzero column is col 0
    (so psum rows 1..127 are exactly 0 -> exp gives 1.0 there), and as
    a [128, 65] AV lhs whose rows 1..127 are zero INCLUDING the ones
    column (so the 127 fake keys contribute exactly nothing to either
    the numerator or the denominator).  temb costs one extra 256-wide
    chunk per stream instead of a full 17th k-block.
  - xa ships as 4 token-chunk tensors [128, 4j, 512] so the projection
    runs all-K per chunk as soon as its DMA lands (proj/transpose
    pipeline with the DMA stream instead of waiting for all of xa).
  - diag trim: of the 4 diagonal k-blocks per (c, t) stream, the two
    half-invisible ones are streamed at N=128 (visible half only).
    exp columns per head-stream drop 2816 -> 2560; one [128, 1536]
    diag-mask multiply per stream (diag chunk is always chunk 0).
  - chunks per stream: [768 diag, 768, 768, 256 temb] per head (psum
    [128, 1536], 3 banks, bufs=2): logits row-pair matmuls -> one wide
    exp ACTIVATE -> AV matmuls, two (c, t) streams interleaved at
    chunk granularity so the PE always has independent work while an
    exp is in flight.
  - ACT does exp only during attention; psum evacuations ride DVE.
"""

import sys

sys.path.insert(0, "/opt/trn_rl_repo")

from contextlib import ExitStack

import ml_dtypes
import numpy as np

import concourse.bass as bass
import concourse.mybir as mybir
import concourse.tile as tile
from concourse import bacc
from concourse.bass import ds, ts
from concourse.bass_utils import run_bass_kernel_spmd
from concourse.masks import make_identity


def _ensure_ntff_hook():
    """This image's antenv lacks axon_hooks; synthesize it so trace=True can
    reach the libaxon NTFF profiler (used by test.py, harmless otherwise)."""
    import types

    try:
        from antenv import axon_hooks  # noqa: F401

        return
    except ImportError:
        pass
    mod = types.ModuleType("antenv.axon_hooks")
    mod._hook = None
    mod.set_axon_ntff_profile_hook = lambda h: setattr(mod, "_hook", h)
    mod.get_axon_ntff_profile_hook = lambda: mod._hook
    import antenv

    sys.modules["antenv.axon_hooks"] = mod
    antenv.axon_hooks = mod
    try:
        from trn_agent_boot.trn_boot import _ntff_profile_via_ctypes

        mod._hook = _ntff_profile_via_ctypes("/opt/axon/libaxon_pjrt.so")
    except Exception:
        pass


_ensure_ntff_hook()

F32 = mybir.dt.float32
BF16 = mybir.dt.bfloat16
AF = mybir.ActivationFunctionType
ALU = mybir.AluOpType

P = 128
S = 1024
D = 512
E = 512
HG = 4  # heads per core
HD = 64
CS = HG * HD  # 256 feature cols per core
QW = 256  # query tile width
CW = 768  # max chunk width per head
CWS = (768, 768, 512, 512)  # per-chunk widths per head
EXB = (0, 1536, 3072, 4096)  # ex col base of each chunk (2 heads packed)
EXW = 2560  # ex cols per head per stream
TNY_AV = 256  # col offset of the temb AV-lhs [128, 4, 65] inside tny
TEMB = 16  # pseudo k-block id for the temb chunk


def _stream_chunks(t):
    """Per (c, t) stream: 4 chunks of blocks (kb, q0, w).  kb indexes kvT
    128-col blocks (l2r 0-7, r2l 8-15; 16 = temb pseudo-block); q0 is the
    offset into the 256-wide q tile; w the streamed width.  Chunk 0 is the
    diag chunk [l2r 2t (tri|full), l2r 2t+1 (tri), r2l 2t (tri), r2l 2t+1
    (full|tri)] so the single mask multiply always hits chunk 0."""
    t2 = 2 * t
    diag = [(t2, 0, 256), (t2 + 1, 128, 128), (8 + t2, 0, 128), (8 + t2 + 1, 0, 256)]
    pure = [(kb, 0, 256) for kb in range(t2)] + [
        (8 + kb, 0, 256) for kb in range(t2 + 2, 8)
    ]
    return [diag, pure[0:3], pure[3:5], pure[5:6] + [(TEMB, 0, 256)]]


def _r(ap):
    return ap.bitcast(mybir.dt.float32r)


def _build_body(ctx, tc):
    nc = tc.nc
    ctx.enter_context(nc.allow_low_precision(reason="bf16 attention pipeline"))

    xa = [
        nc.dram_tensor(f"xa{k}", [P, 4, 512], BF16, kind="ExternalInput").ap()
        for k in range(4)
    ]
    wkT = nc.dram_tensor("wkT", [P, 2, 4, P], BF16, kind="ExternalInput").ap()
    woT = nc.dram_tensor("woT", [HD, HG, E], BF16, kind="ExternalInput").ap()
    tnyT = nc.dram_tensor("tny", [P, 520], BF16, kind="ExternalInput").ap()
    out = nc.dram_tensor("out_part", [S, E], BF16, kind="ExternalOutput").ap()

    inp = ctx.enter_context(tc.tile_pool(name="inp", bufs=1))
    kvp = ctx.enter_context(tc.tile_pool(name="kvp", bufs=1))
    kvag = ctx.enter_context(tc.tile_pool(name="kvag", bufs=1))
    maskp = ctx.enter_context(tc.tile_pool(name="maskp", bufs=1))
    xtp = ctx.enter_context(tc.tile_pool(name="xtp", bufs=1))
    expp = ctx.enter_context(tc.tile_pool(name="expp", bufs=2))
    nrm = ctx.enter_context(tc.tile_pool(name="nrm", bufs=2))
    outp = ctx.enter_context(tc.tile_pool(name="outp", bufs=3))

    # ---- input DMAs.  Weights first (the projection's LDWEIGHTS needs wk),
    # then the four xa token chunks the proj loop consumes in order.
    wk = inp.tile([P, 2, 4, P], BF16)
    allTok = [inp.tile([P, 4, 512], BF16, name=f"allTok{k}") for k in range(4)]
    # wk ships in two contiguous c-halves so the very first proj matmul
    # (c=0) only waits on a 128KB transfer; xa0 is interleaved between
    # them.  tny/wo2 dispatch from the scalar HWDGE queue in parallel.
    nc.sync.dma_start(out=wk[:, 0], in_=wkT[:, 0])
    nc.sync.dma_start(out=allTok[0][:], in_=xa[0])
    nc.sync.dma_start(out=wk[:, 1], in_=wkT[:, 1])
    for k in (2, 1, 3):
        nc.sync.dma_start(out=allTok[k][:], in_=xa[k])
    tny = inp.tile([P, 520], BF16)
    nc.scalar.dma_start(out=tny[:], in_=tnyT)
    wo2 = inp.tile([HD, HG, E], BF16)
    nc.scalar.dma_start(out=wo2[:], in_=woT)

    # ---- diag-chunk mask [128k, 1536]: [tril|ones|tril|triu|ones|triu] x2
    # heads.  t-invariant: the diag chunk layout is the same for every t.
    ones_bf = maskp.tile([P, P], BF16)
    nc.gpsimd.memset(ones_bf[:], 1.0)
    dmask = maskp.tile([P, 2 * CW], BF16)
    nc.vector.memset(dmask[:], 1.0)
    for h0 in (0, CW):
        for off, (pat, cm) in [(0, (1, -1)), (256, (1, -1)), (384, (-1, 1)), (640, (-1, 1))]:
            nc.gpsimd.affine_select(
                dmask[:, ds(h0 + off, P)],
                ones_bf[:],
                pattern=[[pat, P]],
                compare_op=ALU.is_ge,
                fill=0.0,
                base=0,
                channel_multiplier=cm,
            )

    # identity for PE transposes
    ident_stage = maskp.tile([P, P], F32)
    make_identity(nc, ident_stage[:])
    ident = maskp.tile([P, P], BF16)
    nc.vector.tensor_copy(ident[:], ident_stage[:])

    # ones row (bf16) for the denominator-broadcast matmul
    ones_b = maskp.tile([HD + 1, HD], BF16)
    nc.gpsimd.memset(ones_b[:], 1.0)

    # ---- projection + transposes, pipelined with the xa chunk DMAs ----
    # kvT[c] cols: [l2r 0:1024 | r2l 1024:2048]; chunk k covers cols
    # [512k, 512k+512) = key blocks 4k..4k+3.
    kvT = [kvp.tile([P, 2 * S], BF16, name=f"kvT{c}") for c in range(2)]
    qT = [kvp.tile([P, S], BF16, name=f"qT{c}") for c in range(2)]
    kva = kvag.tile([P, 16, HG, HD + 1], BF16)

    with tc.tile_pool(name="ps512", bufs=3, space="PSUM") as ps512, tc.tile_pool(
        name="pst", bufs=3, space="PSUM"
    ) as pst:
        done = set()
        for k in (0, 2, 1, 3):  # l2r0, r2l0, l2r1, r2l1: qT half 0 and the
            # T=0 streams' key blocks are ready after two transfers
            for c in range(2):
                pp = ps512.tile([P, 512], F32, name="pp", tag="pp")
                for j in range(4):
                    nc.tensor.matmul(
                        pp[:],
                        wk[:, c, j, :],
                        allTok[k][:, j, :],
                        start=(j == 0),
                        stop=(j == 3),
                    )
                # evac on DVE only: an ACT-queue copy here would sit ahead
                # of the first exp ACTIVATE and gate it on the whole proj
                nc.vector.tensor_copy(kvT[c][:, ds(512 * k, 512)], pp[:])
            # transposes of this chunk's 4 key blocks -> kva
            for c in range(2):
                tp = pst.tile([P, 4, P], BF16, name="tp", tag="tp")
                for b in range(4):
                    nc.tensor.transpose(
                        tp[:, b, :], kvT[c][:, ds(512 * k + P * b, P)], ident[:]
                    )
                nc.vector.tensor_copy(
                    kva[:, ds(4 * k, 4), 2 * c : 2 * c + 2, 0:HD],
                    tp[:].rearrange("p a (b c) -> p a b c", b=2),
                )
            done.add(k)
            for half in range(2):
                if half + 2 in done and half in done and (half, "q") not in done:
                    done.add((half, "q"))
                    for c in range(2):
                        nc.vector.tensor_add(
                            qT[c][:, ds(512 * half, 512)],
                            kvT[c][:, ds(512 * half, 512)],
                            kvT[c][:, ds(512 * (half + 2), 512)],
                        )
        nc.vector.memset(kva[:, :, :, HD : HD + 1], 1.0)

    # ---- attention: 8 (c, t) streams, interleaved in pairs ----
    xt = [xtp.tile([HD, S], BF16, name=f"xt{h}") for h in range(HG)]

    with tc.tile_pool(name="lgp", bufs=2, space="PSUM") as lgp, tc.tile_pool(
        name="xpsp", bufs=2, space="PSUM"
    ) as xpsp:

        def denom_copy(p):
            """denom row -> SBUF (DVE), emitted inline right after the
            stream's last AV so it is long done before normalize's PE
            broadcast matmul pops during the next t.  bf16: the
            denominator only needs ~3 digits and bf16 rhs streams 2x."""
            p.cs = nrm.tile([HD + 1, 2 * QW], BF16, name="cs", tag="cs")
            nc.vector.tensor_copy(
                p.cs[HD : HD + 1, :], p.xps[HD : HD + 1, 0 : 2 * QW]
            )

        def normalize(p):
            """PE ones-matmul broadcast into a spare lg slot, reciprocal,
            multiply into xt."""
            c, t = p.c, p.t
            bc = lgp.tile([P, 2 * CW], F32, name="bc", tag="lg")
            for half in range(2):  # matmul out must stay within a psum bank
                nc.tensor.matmul(
                    bc[0:HD, ds(512 * half, 512)],
                    ones_b[HD : HD + 1, :],
                    p.cs[HD : HD + 1, ds(512 * half, 512)],
                    start=True,
                    stop=True,
                )
            bcs = nrm.tile([HD, 2 * QW], F32, name="bcs", tag="bcs")
            nc.vector.reciprocal_approx_fast(bcs[:], bc[0:HD, 0 : 2 * QW])
            for hh in range(2):
                nc.vector.tensor_mul(
                    xt[2 * c + hh][:, ds(t * QW, QW)],
                    p.xps[0:HD, ds(hh * QW, QW)],
                    bcs[:, ds(hh * QW, QW)],
                )

        pfts = {}

        def outprojA(sp):
            """First half of the out projection for s-block pair sp
            (s-blocks 2sp, 2sp+1): accumulate the c=0 heads.  Only needs
            normalize(pa), so it overlaps the c=1 stream's tail work."""
            pft = lgp.tile([P, 2 * CW], F32, name="pft", tag="lg")
            pfts[sp] = pft
            for i in range(2):
                st = 2 * sp + i
                for h in range(2):
                    nc.tensor.matmul(
                        pft[:, ds(512 * i, E)],
                        xt[h][:, ts(st, P)],
                        wo2[:, h, :],
                        start=(h == 0),
                        stop=False,
                    )

        def outprojB(sp):
            """Second half: accumulate the c=1 heads, evacuate on DVE (ACT
            is saturated by exp), DMA out across both HWDGE queues."""
            pft = pfts.pop(sp)
            for i in range(2):
                st = 2 * sp + i
                for h in range(2, HG):
                    nc.tensor.matmul(
                        pft[:, ds(512 * i, E)],
                        xt[h][:, ts(st, P)],
                        wo2[:, h, :],
                        start=False,
                        stop=(h == 3),
                    )
            for i in range(2):
                st = 2 * sp + i
                ob = outp.tile([P, E], BF16, name="ob", tag="ob")
                nc.vector.tensor_copy(ob[:], pft[:, ds(512 * i, E)])
                if st >= 4:
                    nc.sync.dma_start(out=out[ds(P * st, 64), :], in_=ob[0:64, :])
                    nc.scalar.dma_start(
                        out=out[ds(P * st + 64, 64), :], in_=ob[64:P, :]
                    )
                else:
                    nc.sync.dma_start(out=out[ts(st, P), :], in_=ob[:])

        class Pair:
            """Emission state for one (c, t) stream (2 heads row-paired)."""

            def __init__(self, c, t):
                self.c, self.t = c, t
                self.chunks = _stream_chunks(t)
                self.ex = expp.tile([P, 2 * EXW], BF16, name="ex", tag="ex")
                self.xps = None
                self.cs = None
                self.first_av = [True, True]

            def ex_off(self, ci, hh):
                """ex col offset of (chunk ci, head hh): [E cw | O cw] per
                chunk.  Chunk widths 768/768/512/512 keep the E and O halves
                of every row-paired matmul in different psum banks."""
                return EXB[ci] + hh * CWS[ci]

            def chunk(self, ci):
                c, t = self.c, self.t
                cw = CWS[ci]
                lg = lgp.tile([P, 2 * CW], F32, name="lg", tag="lg")
                off = 0
                for kb, q0, w in self.chunks[ci]:
                    for hh in range(2):
                        lhs = (
                            tny[ds(HD * hh, HD), ds(P * c, P)]
                            if kb == TEMB
                            else kvT[c][ds(HD * hh, HD), ds(P * kb, P)]
                        )
                        nc.tensor.matmul(
                            lg[:, ds(hh * cw + off, w)],
                            lhs,
                            qT[c][ds(HD * hh, HD), ds(QW * t + q0, w)],
                            start=True,
                            stop=True,
                        )
                    off += w
                nc.scalar.activation(
                    self.ex[:, ds(EXB[ci], 2 * cw)],
                    lg[:, 0 : 2 * cw],
                    AF.Exp,
                    scale=0.125,
                )
                if ci == 0:
                    nc.vector.tensor_mul(
                        self.ex[:, 0 : 2 * CW], self.ex[:, 0 : 2 * CW], dmask[:]
                    )

            def emit_av(self, ci):
                if self.xps is None:
                    self.xps = xpsp.tile([P, 2 * QW], F32, name="xps", tag="xps")
                off = 0
                nblk = len(self.chunks[ci])
                for bi, (kb, q0, w) in enumerate(self.chunks[ci]):
                    for hh in range(2):
                        lhs = (
                            tny[:, ds(TNY_AV + 65 * (2 * self.c + hh), 65)]
                            if kb == TEMB
                            else kva[:, kb, 2 * self.c + hh, :]
                        )
                        nc.tensor.matmul(
                            self.xps[0 : HD + 1, ds(QW * hh + q0, w)],
                            lhs,
                            self.ex[:, ds(self.ex_off(ci, hh) + off, w)],
                            start=self.first_av,
                            stop=(ci == 3 and bi == nblk - 1 and hh == 1),
                        )
                        self.first_av = False
                    off += w

        pending = []

        def drain():
            if pending:
                pending.pop(0)()

        for t in range(4):
            pa, pb = Pair(0, t), Pair(1, t)
            for ci in range(4):
                pa.chunk(ci)
                drain()
                pb.chunk(ci)
                drain()
                if ci > 0:
                    pa.emit_av(ci - 1)
                    pb.emit_av(ci - 1)
            pa.emit_av(3)
            denom_copy(pa)
            if t == 3:
                # final t: flush eagerly so the tail normalize overlaps
                # pb's remaining AV matmuls
                normalize(pa)
                pb.emit_av(3)
                denom_copy(pb)
                normalize(pb)
                pending.append(lambda st=2 * t: outproj(st))
                pending.append(lambda st=2 * t + 1: outproj(st))
            else:
                pb.emit_av(3)
                denom_copy(pb)
                pending.append(lambda p=pa: normalize(p))
                pending.append(lambda p=pb: normalize(p))
                pending.append(lambda st=2 * t: outproj(st))
                pending.append(lambda st=2 * t + 1: outproj(st))
        for fn in pending:
            fn()


_NC_CACHE = None


def build_nc():
    global _NC_CACHE
    if _NC_CACHE is None:
        nc = bacc.Bacc(
            "TRN2",
            target_bir_lowering=False,
            debug=False,
            num_devices=8,
        )
        with tile.TileContext(nc) as tc, ExitStack() as ctx:
            _build_body(ctx, tc)
        nc.compile()
        _NC_CACHE = nc
    return _NC_CACHE


def _bf16(x):
    return np.ascontiguousarray(x).astype(ml_dtypes.bfloat16)


def make_in_maps(l2r_embed, r2l_embed, temb, W_dense, W_out):
    in_maps = []
    B = l2r_embed.shape[0]
    # xa chunks [p, j, n] = xaT[128j+p, 512k+n], xaT = [l2r.T | r2l.T]
    xac = {}
    for b in range(B):
        xaT = np.concatenate([l2r_embed[b].T, r2l_embed[b].T], axis=1)
        x4 = xaT.reshape(4, P, 4, 512)
        xac[b] = [_bf16(x4[:, :, k, :].transpose(1, 0, 2)) for k in range(4)]
    for core in range(8):
        b, hg = core // 2, core % 2
        cols = slice(CS * hg, CS * (hg + 1))
        # [p, c, j, m]: wkh[p, c, j, m] = W_dense.T[128j+p, 128c+m]
        wkh = (
            W_dense[cols, :].T.reshape(4, P, 2, P).transpose(1, 2, 0, 3)
        )
        woh = W_out[:, cols].T.reshape(HG, HD, E).transpose(1, 0, 2)
        kvt = W_dense[cols, :] @ temb[b]  # [256] f32
        tny = np.zeros((P, 520), np.float32)
        # temb logits lhs: [128, 128] per c, only col 0 nonzero
        for c in range(2):
            tny[:, P * c] = kvt[P * c : P * c + P]
        # temb AV lhs: [128, 65] per head, only row 0 nonzero (incl ones col)
        for h in range(HG):
            tny[0, TNY_AV + 65 * h : TNY_AV + 65 * h + HD] = kvt[
                HD * h : HD * h + HD
            ]
            tny[0, TNY_AV + 65 * h + HD] = 1.0
        m = {f"xa{k}": xac[b][k] for k in range(4)}
        m.update({"wkT": _bf16(wkh), "woT": _bf16(woh), "tny": _bf16(tny)})
        in_maps.append(m)
    return in_maps


def kernel(l2r_embed, r2l_embed, temb, W_dense, W_out, b_out, num_heads, **run_kwargs):
    assert int(num_heads) == 8
    l2r_embed = np.asarray(l2r_embed, np.float32)
    r2l_embed = np.asarray(r2l_embed, np.float32)
    temb = np.asarray(temb, np.float32)
    W_dense = np.asarray(W_dense, np.float32)
    W_out = np.asarray(W_out, np.float32)
    b_out = np.asarray(b_out, np.float32)

    nc = build_nc()
    in_maps = make_in_maps(l2r_embed, r2l_embed, temb, W_dense, W_out)
    res = run_bass_kernel_spmd(nc, in_maps, core_ids=list(range(8)), **run_kwargs)

    B = l2r_embed.shape[0]
    outp = np.empty((B, S, E), np.float32)
    for b in range(B):
        outp[b] = (
            res.results[2 * b]["out_part"].astype(np.float32)
            + res.results[2 * b + 1]["out_part"].astype(np.float32)
            + b_out[None, :]
        )
    if run_kwargs:
        kernel.last_results = res
    return outp
